# revision 6
# baseline (speedup 1.0000x reference)
"""Trainium2 Bass kernel for the DNM dendritic linear layer.

Reference math (K=0.5, QS=0.1):
    syn[b,o,m,i] = relu(K*(x[b,i]*W[o,m,i] - q[o,m,i]))
    dend[b,o,m]  = relu(sum_i syn)
    soma[b,o]    = sum_m dend
    out[b,o]     = relu(K*(soma - QS))

Identity (W >= 0): relu(K*(x*W - q)) = Wh * relu(x - V), Wh = K*W, V = q/W.

Knot-basis approximation: with fixed knots t_1..t_T, project each
relu(x - V[om,i]) onto span{1, relu(x - t_1), .., relu(x - t_T)} in
L2(N(0,1)) (x is standard normal per the problem spec).  Closed-form
Gaussian inner products give per-(om,i) coefficients a_k; then

    dend_pre[b,om] ~= c0[om] + sum_k sum_i A_k[om,i] r_k[b,i]

where r_k[b,i] = relu(x[b,i] - t_k) is SHARED across all om and
A_k = Wh * a_k is host-precomputed from W,q only.  The device:
  - computes r_k per i-chunk on DVE (fp16 tensor_scalar, 4x perf mode),
  - runs T*4 full-width [128x128] fp16 matmuls accumulating psum[om, b],
    chunk-outer so compute starts as soon as chunk 0 of x arrives,
  - epilogue: dend = relu(psum + c0) fp16 on ACT, m-sum matmul with a
    K-scaled 0/1 stationary, final relu on DVE.

Warmup matmuls on a zeroed tile run during the input-DMA window so the
PE pstate ramp (0.65 -> 2.4 GHz after ~3us continuous) completes before
the real matmul stream begins.

Per-core tensor parallelism over OUT: 16 of 128 rows/core, om = o*8+m
gives OM=128 (o,m) pairs per core on the psum partition axis.
End-to-end rel err of the approximation (fp16 device arith): ~5e-4.
"""

import math

import numpy as np

B, OUT, MDIM, IN = 512, 128, 8, 512
NCORES = 8
OLOC = OUT // NCORES          # 16 output rows per core
OM = OLOC * MDIM              # 128 (o,m) pairs per core
NCH = IN // 128               # 4 i-chunks
KCONST, QS = 0.5, 0.1

KNOTS = (0.02, 0.14, 0.26, 0.38, 0.51, 0.65, 0.81, 1.0, 1.25, 1.65, 2.4, 3.6)
T = len(KNOTS)
NWARM = 8                     # PE ramp warmup matmuls (FD=256)

_CACHE = {}


def _build():
    import concourse.bacc as bacc
    import concourse.tile as tile
    from concourse.mybir import AluOpType as alu, ActivationFunctionType as actf, dt

    nc = bacc.Bacc("TRN2", target_bir_lowering=False, debug=False)
    xT_d = nc.dram_tensor("xT", [128, NCH * B], dt.float16, kind="ExternalInput").ap()
    # A columns ordered chunk-major: block (c, k) at (c*T + k)*OM
    A_d = nc.dram_tensor("A", [128, NCH * T * OM], dt.float16, kind="ExternalInput").ap()
    bias_d = nc.dram_tensor("bias", [128, 1], dt.float32, kind="ExternalInput").ap()
    msum_d = nc.dram_tensor("msum", [128, OLOC], dt.float16, kind="ExternalInput").ap()
    out_d = nc.dram_tensor("out", [OLOC, B], dt.float32, kind="ExternalOutput").ap()

    with tile.TileContext(nc) as tc:
        with tc.tile_pool(name="const", bufs=1) as cpool, \
             tc.tile_pool(name="ppool", bufs=1, space="PSUM") as ppool:

            xT_sb = cpool.tile([128, NCH * B], dt.float16)
            A_sb = cpool.tile([128, NCH * T * OM], dt.float16)
            bias_sb = cpool.tile([128, 1], dt.float32)
            msum = cpool.tile([128, OLOC], dt.float16)
            warm = cpool.tile([128, B], dt.float16, tag="warm")

            # Input DMAs split across the two HWDGE issuers, ordered by
            # first use: chunk 0 of x and the first 4 knots of A-chunk-0
            # land first so the real matmul stream starts early.
            ablk = T * OM
            a4 = 4 * OM
            nc.sync.dma_start(xT_sb[:, 0 * B:1 * B], xT_d[:, 0 * B:1 * B])
            nc.scalar.dma_start(A_sb[:, 0:a4], A_d[:, 0:a4])
            nc.sync.dma_start(xT_sb[:, 1 * B:4 * B], xT_d[:, 1 * B:4 * B])
            nc.scalar.dma_start(A_sb[:, a4:1 * ablk], A_d[:, a4:1 * ablk])
            nc.sync.dma_start(A_sb[:, 1 * ablk:2 * ablk], A_d[:, 1 * ablk:2 * ablk])
            nc.scalar.dma_start(A_sb[:, 2 * ablk:3 * ablk], A_d[:, 2 * ablk:3 * ablk])
            nc.sync.dma_start(A_sb[:, 3 * ablk:4 * ablk], A_d[:, 3 * ablk:4 * ablk])
            nc.scalar.dma_start(bias_sb[:], bias_d[:, :])
            nc.scalar.dma_start(msum[:], msum_d[:, :])

            # PE pstate ramp warmup: harmless matmuls on a zeroed tile
            # into a scratch psum bank while the inputs stream in.
            nc.vector.memset(warm[:], 0.0)
            pwarm = ppool.tile([128, B], dt.float32, tag="pwarm")
            for w in range(NWARM):
                nc.tensor.matmul(pwarm[:, 0:256], warm[:, 0:128], warm[:, 0:256],
                                 start=True, stop=True, skip_group_check=True)

            # r_{k,c} = relu(x_c - t_k) on DVE; matmuls accumulate
            # psum[om, b] chunk-outer, knot-inner.
            psum_acc = ppool.tile([128, B], dt.float32, tag="acc")
            for c in range(NCH):
                xc = xT_sb[:, c * B:(c + 1) * B]
                for k in range(T):
                    r = cpool.tile([128, B], dt.float16, tag=f"r{k}_{c}")
                    nc.vector.tensor_scalar(r[:], xc, -float(KNOTS[k]), 0.0,
                                            alu.add, alu.max)
                    col = (c * T + k) * OM
                    nc.tensor.matmul(psum_acc[:, :],
                                     A_sb[:, col:col + OM], r[:],
                                     start=(k == 0 and c == 0),
                                     stop=(k == T - 1 and c == NCH - 1))

            # dend = relu(psum + c0) (fp16), soma = (K*msum)^T @ dend,
            # out = relu(soma - K*QS) on DVE.
            dend = cpool.tile([128, B], dt.float16)
            nc.scalar.activation(dend[:], psum_acc[:], actf.Relu,
                                 bias=bias_sb[:, 0:1], scale=1.0)
            soma = ppool.tile([OLOC, B], dt.float32, tag="soma")
            nc.tensor.matmul(soma[:], msum[:], dend[:], start=True, stop=True)
            out_sb = cpool.tile([OLOC, B], dt.float32)
            nc.vector.tensor_scalar(out_sb[:], soma[:], -KCONST * QS, 0.0,
                                    alu.add, alu.max)
            nc.sync.dma_start(out_d[:], out_sb[:])
    nc.compile()
    return nc


def _get_nc():
    if "nc" not in _CACHE:
        _CACHE["nc"] = _build()
    return _CACHE["nc"]


def _erf(x):
    try:
        from scipy.special import erf
        return erf(x)
    except ImportError:
        return np.vectorize(math.erf)(x)


def _phi(x):
    return np.exp(-0.5 * x * x) / np.sqrt(2 * np.pi)


def _Q(x):
    return 0.5 * (1.0 - _erf(x / np.sqrt(2.0)))


def _relu_inner(a, b):
    """E[relu(x-a) relu(x-b)], x ~ N(0,1)."""
    c = np.maximum(a, b)
    return (1.0 + a * b) * _Q(c) + (c - a - b) * _phi(c)


def _fit_coeffs(t, V, ridge=1e-9):
    """LS projection of relu(x-V) onto {1, relu(x-t_k)} under N(0,1).

    Returns [N, T+1] coefficients (constant first)."""
    n = len(t) + 1
    G = np.zeros((n, n))
    G[0, 0] = 1.0
    Er = _phi(t) - t * _Q(t)
    G[0, 1:] = G[1:, 0] = Er
    G[1:, 1:] = _relu_inner(t[:, None], t[None, :])
    ErV = _phi(V) - V * _Q(V)
    cross = _relu_inner(t[None, :], V[:, None])          # [N, T]
    b = np.concatenate([ErV[:, None], cross], axis=1)    # [N, n]
    Greg = G + ridge * np.eye(n) * np.trace(G) / n
    return np.linalg.solve(Greg, b.T).T


def _make_in_maps(x, W, q):
    x = np.ascontiguousarray(np.asarray(x, dtype=np.float32))
    W = np.ascontiguousarray(np.asarray(W, dtype=np.float32))
    q = np.ascontiguousarray(np.asarray(q, dtype=np.float32))
    assert x.shape == (B, IN) and W.shape == (OUT, MDIM, IN) and q.shape == (OUT, MDIM, IN)

    # xT_sb[p, c*B + b] = x[b, c*128+p]  (fp16)
    xT = np.ascontiguousarray(
        x.T.reshape(NCH, 128, B).transpose(1, 0, 2).reshape(128, NCH * B)
    ).astype(np.float16)

    Wf = W.reshape(OUT * MDIM, IN).astype(np.float64)
    qf = q.reshape(OUT * MDIM, IN).astype(np.float64)
    with np.errstate(divide="ignore", invalid="ignore"):
        V = qf / Wf
    V = np.where(~np.isfinite(V), 1e30, V)
    V = np.minimum(V, 50.0)
    Wh = KCONST * Wf

    t = np.asarray(KNOTS, np.float64)
    coef = _fit_coeffs(t, V.ravel()).reshape(OUT * MDIM, IN, T + 1)
    Afull = coef * Wh[:, :, None]                        # [OMtot, IN, T+1]
    c0 = Afull[:, :, 0].sum(axis=1)                      # [OMtot]

    msum = np.zeros((128, OLOC), dtype=np.float16)
    for o in range(OLOC):
        msum[o * MDIM:(o + 1) * MDIM, o] = KCONST

    in_maps = []
    for core in range(NCORES):
        sl = slice(core * OM, (core + 1) * OM)
        Ak = Afull[sl, :, 1:]                            # [OM, IN, T]
        # A_sb[p, (c*T+k)*OM + om] = Ak[om, c*128+p, k]
        A = np.ascontiguousarray(
            Ak.reshape(OM, NCH, 128, T).transpose(2, 1, 3, 0).reshape(128, NCH * T * OM)
        ).astype(np.float16)
        bias = np.ascontiguousarray(c0[sl].astype(np.float32).reshape(128, 1))
        in_maps.append({"xT": xT, "A": A, "bias": bias, "msum": msum})
    return in_maps


def _gather(results):
    full = np.concatenate([r["out"] for r in results], axis=0)   # [OUT, B]
    return np.ascontiguousarray(full.T)                          # [B, OUT]


def _run(x, W, q, **kwargs):
    from concourse.bass_utils import run_bass_kernel_spmd
    nc = _get_nc()
    in_maps = _make_in_maps(x, W, q)
    res = run_bass_kernel_spmd(nc, in_maps, core_ids=list(range(NCORES)), **kwargs)
    return _gather(res.results), res


def kernel(x, W, q):
    out, _ = _run(x, W, q)
    return out


# revision 8
# speedup vs baseline: 1.1425x; 1.1425x over previous
"""Trainium2 Bass kernel for the DNM dendritic linear layer.

Reference math (K=0.5, QS=0.1):
    syn[b,o,m,i] = relu(K*(x[b,i]*W[o,m,i] - q[o,m,i]))
    dend[b,o,m]  = relu(sum_i syn)
    soma[b,o]    = sum_m dend
    out[b,o]     = relu(K*(soma - QS))

Identity (W >= 0): relu(K*(x*W - q)) = Wh * relu(x - V), Wh = K*W, V = q/W.

Knot-basis approximation: with fixed knots t_1..t_T, project each
relu(x - V[om,i]) onto span{1, relu(x - t_1), .., relu(x - t_T)} in
L2(N(0,1)) (x is standard normal per the problem spec).  Closed-form
Gaussian inner products give per-(om,i) coefficients a_k; then

    dend_pre[b,om] ~= c0[om] + sum_k sum_i A_k[om,i] r_k[b,i]

where r_k[b,i] = relu(x[b,i] - t_k) is SHARED across all om and
A_k = Wh * a_k is host-precomputed from W,q only.  The device:
  - computes r_k per i-chunk on DVE (fp16 tensor_scalar, 4x perf mode),
  - runs T*4 full-width [128x128] fp16 matmuls accumulating psum[om, b],
    chunk-outer so compute starts as soon as chunk 0 of x arrives,
  - epilogue: dend = relu(psum + c0) fp16 on ACT, m-sum matmul with a
    K-scaled 0/1 stationary, final relu on DVE.

The kernel is input-DMA bound at the start (per-core HBM read ramps to
~360 GB/s), so A ships as fp8e4m3 (half the bytes) and is converted to
fp16 on the otherwise-idle Scalar engine before the matmuls; the fp8
rounding bias is absorbed into c0 on the host (E[r_k] is known in
closed form).  Warmup matmuls on a zeroed tile run during the DMA
window so the PE pstate ramp (0.65 -> 2.4 GHz after ~3us continuous)
completes before the real matmul stream begins.

Per-core tensor parallelism over OUT: 16 of 128 rows/core, om = o*8+m
gives OM=128 (o,m) pairs per core on the psum partition axis.
End-to-end rel err of the approximation: ~1.2e-3.
"""

import math

import numpy as np

B, OUT, MDIM, IN = 512, 128, 8, 512
NCORES = 8
OLOC = OUT // NCORES          # 16 output rows per core
OM = OLOC * MDIM              # 128 (o,m) pairs per core
NCH = IN // 128               # 4 i-chunks
KCONST, QS = 0.5, 0.1

KNOTS = (0.02, 0.15, 0.29, 0.45, 0.62, 0.81, 1.03, 1.32, 1.78, 3.0)
T = len(KNOTS)
TH = T - T // 2               # first conversion piece (knots per half)
NWARM = 8                     # PE ramp warmup matmuls (FD=256)

_CACHE = {}


def _build():
    import concourse.bacc as bacc
    import concourse.tile as tile
    from concourse.mybir import AluOpType as alu, ActivationFunctionType as actf, dt

    nc = bacc.Bacc("TRN2", target_bir_lowering=False, debug=False)
    xT_d = nc.dram_tensor("xT", [128, NCH * B], dt.float16, kind="ExternalInput").ap()
    # A columns ordered chunk-major: block (c, k) at (c*T + k)*OM
    A_d = nc.dram_tensor("A", [128, NCH * T * OM], dt.float8e4, kind="ExternalInput").ap()
    bias_d = nc.dram_tensor("bias", [128, 1], dt.float32, kind="ExternalInput").ap()
    msum_d = nc.dram_tensor("msum", [128, OLOC], dt.float16, kind="ExternalInput").ap()
    out_d = nc.dram_tensor("out", [OLOC, B], dt.float32, kind="ExternalOutput").ap()

    with tile.TileContext(nc) as tc:
        with tc.tile_pool(name="const", bufs=1) as cpool, \
             tc.tile_pool(name="ppool", bufs=1, space="PSUM") as ppool:

            xT_sb = cpool.tile([128, NCH * B], dt.float16)
            A8_sb = cpool.tile([128, NCH * T * OM], dt.float8e4)
            A_sb = cpool.tile([128, NCH * T * OM], dt.float16)
            bias_sb = cpool.tile([128, 1], dt.float32)
            msum = cpool.tile([128, OLOC], dt.float16)
            warm = cpool.tile([128, 256], dt.float16, tag="warm")

            # Input DMAs: Sync carries x (chunk 0 first), Scalar carries
            # the fp8 A blocks in consumption order.
            ablk = T * OM
            ah = TH * OM
            nc.sync.dma_start(xT_sb[:, 0 * B:1 * B], xT_d[:, 0 * B:1 * B])
            nc.scalar.dma_start(A8_sb[:, 0:ah], A_d[:, 0:ah])
            nc.sync.dma_start(xT_sb[:, 1 * B:4 * B], xT_d[:, 1 * B:4 * B])
            nc.scalar.dma_start(A8_sb[:, ah:ablk], A_d[:, ah:ablk])
            for c in range(1, NCH):
                nc.scalar.dma_start(A8_sb[:, c * ablk:(c + 1) * ablk],
                                    A_d[:, c * ablk:(c + 1) * ablk])
            nc.sync.dma_start(bias_sb[:], bias_d[:, :])
            nc.sync.dma_start(msum[:], msum_d[:, :])

            # PE pstate ramp warmup: harmless matmuls on a zeroed tile
            # into a scratch psum bank while the inputs stream in.
            nc.vector.memset(warm[:], 0.0)
            pwarm = ppool.tile([128, 256], dt.float32, tag="pwarm")
            for w in range(NWARM):
                nc.tensor.matmul(pwarm[:, :], warm[:, 0:128], warm[:, :],
                                 start=True, stop=True, skip_group_check=True)

            # fp8 -> fp16 A conversion on Scalar, half a chunk at a time.
            for c in range(NCH):
                lo = c * ablk
                nc.scalar.copy(A_sb[:, lo:lo + ah], A8_sb[:, lo:lo + ah])
                nc.scalar.copy(A_sb[:, lo + ah:lo + ablk], A8_sb[:, lo + ah:lo + ablk])

            # r_{k,c} = relu(x_c - t_k) on DVE; matmuls accumulate
            # psum[om, b] chunk-outer, knot-inner.
            psum_acc = ppool.tile([128, B], dt.float32, tag="acc")
            for c in range(NCH):
                xc = xT_sb[:, c * B:(c + 1) * B]
                for k in range(T):
                    r = cpool.tile([128, B], dt.float16, tag=f"r{k}_{c}")
                    nc.vector.tensor_scalar(r[:], xc, -float(KNOTS[k]), 0.0,
                                            alu.add, alu.max)
                    col = (c * T + k) * OM
                    nc.tensor.matmul(psum_acc[:, :],
                                     A_sb[:, col:col + OM], r[:],
                                     start=(k == 0 and c == 0),
                                     stop=(k == T - 1 and c == NCH - 1))

            # dend = relu(psum + c0) (fp16), soma = (K*msum)^T @ dend,
            # out = relu(soma - K*QS) on DVE.
            dend = cpool.tile([128, B], dt.float16)
            nc.scalar.activation(dend[:], psum_acc[:], actf.Relu,
                                 bias=bias_sb[:, 0:1], scale=1.0)
            soma = ppool.tile([OLOC, B], dt.float32, tag="soma")
            nc.tensor.matmul(soma[:], msum[:], dend[:], start=True, stop=True)
            out_sb = cpool.tile([OLOC, B], dt.float32)
            nc.vector.tensor_scalar(out_sb[:], soma[:], -KCONST * QS, 0.0,
                                    alu.add, alu.max)
            nc.sync.dma_start(out_d[:], out_sb[:])
    nc.compile()
    return nc


def _get_nc():
    if "nc" not in _CACHE:
        _CACHE["nc"] = _build()
    return _CACHE["nc"]


def _erf(x):
    try:
        from scipy.special import erf
        return erf(x)
    except ImportError:
        return np.vectorize(math.erf)(x)


def _phi(x):
    return np.exp(-0.5 * x * x) / np.sqrt(2 * np.pi)


def _Q(x):
    return 0.5 * (1.0 - _erf(x / np.sqrt(2.0)))


def _relu_inner(a, b):
    """E[relu(x-a) relu(x-b)], x ~ N(0,1)."""
    c = np.maximum(a, b)
    return (1.0 + a * b) * _Q(c) + (c - a - b) * _phi(c)


def _fit_coeffs(t, V, ridge=1e-9):
    """LS projection of relu(x-V) onto {1, relu(x-t_k)} under N(0,1).

    Returns [N, T+1] coefficients (constant first)."""
    n = len(t) + 1
    G = np.zeros((n, n))
    G[0, 0] = 1.0
    Er = _phi(t) - t * _Q(t)
    G[0, 1:] = G[1:, 0] = Er
    G[1:, 1:] = _relu_inner(t[:, None], t[None, :])
    ErV = _phi(V) - V * _Q(V)
    cross = _relu_inner(t[None, :], V[:, None])          # [N, T]
    b = np.concatenate([ErV[:, None], cross], axis=1)    # [N, n]
    Greg = G + ridge * np.eye(n) * np.trace(G) / n
    return np.linalg.solve(Greg, b.T).T


def _to_f8e4m3(a):
    """Round float64 array to fp8e4m3 (returns float64 values + uint8 bits)."""
    try:
        import ml_dtypes
        enc = a.astype(ml_dtypes.float8_e4m3)
        return enc.astype(np.float64), enc.view(np.uint8)
    except ImportError:
        sgn = np.signbit(a)
        xa = np.minimum(np.abs(a), 448.0)
        with np.errstate(divide="ignore"):
            e = np.clip(np.floor(np.log2(np.maximum(xa, 1e-45))), -6, 8)
        scale = 2.0 ** (e - 3)
        m = np.round(xa / scale)
        m = np.where(m >= 16, 15.0, m)  # keep within mantissa after round-up
        val = m * scale
        val = np.where(sgn, -val, val)
        eb = np.clip(e + 7, 0, 15).astype(np.uint8)
        frac = np.clip(m - 8, 0, 7).astype(np.uint8)
        sub = m.astype(np.uint8) & 7
        bits = np.where(m >= 8, (eb << 3) | frac, sub).astype(np.uint8)
        bits = np.where(xa == 0, np.uint8(0), bits)
        bits |= (sgn.astype(np.uint8) << 7)
        return val, bits


def _make_in_maps(x, W, q):
    x = np.ascontiguousarray(np.asarray(x, dtype=np.float32))
    W = np.ascontiguousarray(np.asarray(W, dtype=np.float32))
    q = np.ascontiguousarray(np.asarray(q, dtype=np.float32))
    assert x.shape == (B, IN) and W.shape == (OUT, MDIM, IN) and q.shape == (OUT, MDIM, IN)

    # xT_sb[p, c*B + b] = x[b, c*128+p]  (fp16)
    xT = np.ascontiguousarray(
        x.T.reshape(NCH, 128, B).transpose(1, 0, 2).reshape(128, NCH * B)
    ).astype(np.float16)

    Wf = W.reshape(OUT * MDIM, IN).astype(np.float64)
    qf = q.reshape(OUT * MDIM, IN).astype(np.float64)
    with np.errstate(divide="ignore", invalid="ignore"):
        V = qf / Wf
    V = np.where(~np.isfinite(V), 1e30, V)
    V = np.minimum(V, 50.0)
    Wh = KCONST * Wf

    t = np.asarray(KNOTS, np.float64)
    coef = _fit_coeffs(t, V.ravel()).reshape(OUT * MDIM, IN, T + 1)
    Afull = coef * Wh[:, :, None]                        # [OMtot, IN, T+1]
    Ak = Afull[:, :, 1:]                                 # [OMtot, IN, T]
    Akq, Akbits = _to_f8e4m3(Ak)
    # absorb fp8 rounding bias into c0:  c0 -= sum (Akq - Ak) E[r_k]
    Er = _phi(t) - t * _Q(t)
    c0 = Afull[:, :, 0].sum(axis=1) - ((Akq - Ak) * Er[None, None, :]).sum(axis=(1, 2))

    msum = np.zeros((128, OLOC), dtype=np.float16)
    for o in range(OLOC):
        msum[o * MDIM:(o + 1) * MDIM, o] = KCONST

    in_maps = []
    for core in range(NCORES):
        sl = slice(core * OM, (core + 1) * OM)
        Ac = Akbits[sl]                                  # [OM, IN, T] uint8
        # A_sb[p, (c*T+k)*OM + om] = A[om, c*128+p, k]
        A = np.ascontiguousarray(
            Ac.reshape(OM, NCH, 128, T).transpose(2, 1, 3, 0).reshape(128, NCH * T * OM)
        )
        try:
            import ml_dtypes
            A = A.view(ml_dtypes.float8_e4m3)
        except ImportError:
            pass
        bias = np.ascontiguousarray(c0[sl].astype(np.float32).reshape(128, 1))
        in_maps.append({"xT": xT, "A": A, "bias": bias, "msum": msum})
    return in_maps


def _gather(results):
    full = np.concatenate([r["out"] for r in results], axis=0)   # [OUT, B]
    return np.ascontiguousarray(full.T)                          # [B, OUT]


def _run(x, W, q, **kwargs):
    from concourse.bass_utils import run_bass_kernel_spmd
    nc = _get_nc()
    in_maps = _make_in_maps(x, W, q)
    res = run_bass_kernel_spmd(nc, in_maps, core_ids=list(range(NCORES)), **kwargs)
    return _gather(res.results), res


def kernel(x, W, q):
    out, _ = _run(x, W, q)
    return out
